# revision 1
# baseline (speedup 1.0000x reference)
"""Trainium2 Bass kernel for intra-segment KNN (K=64 neighbours + self).

Problem: coordinates [32768, 4] f32 split into 8 equal segments (events) of
4096 points; per point, find the 65 nearest points (incl. self) within its
segment, returning (idx int32 [32768,65], dist f32 [32768,65]) sorted by
ascending squared distance, ties broken by lower index (matching
jax.lax.top_k on -d2).

Sharding: one event per NeuronCore (8 cores), pure data parallel.

Per-core algorithm (S=4096 points, D=4 dims):
  - negkey[r, j] = -d2[r, j] computed by TensorE:  psum = 2*c_r.c_j - |c_j|^2
    (contraction dim 8: lhsT rows 0-3 = 2*c^T, rows 4-7 = -1; rhs rows 0-3 =
    c^T, rows 4-7 = (c^T)^2), then ScalarE adds per-row bias -|c_r|^2 while
    copying PSUM -> SBUF.
  - selection per 128-row tile on VectorE with Max8/MaxIndex8/MatchReplace8:
      group phase: 32 groups of 128 columns, keep top-16 (values + local idx)
        -> C [128, 512].  (P(any group holds >16 of the true top-65) ~ 1e-9/row)
      C phase: 9 rounds of (max8 + max_index + match_replace) over C
        -> top-72 values V + their C-slots posC.
  - index unscramble via two GpSimd per-partition local_scatters:
      W2[p, posC[p,k]] = k+1 ; Y[p, W2[p,q]-1] = (local_idx + group_offset)[p,q]
    giving Y[p,k] = column index of k-th nearest neighbour.
  - dist = Relu(-V) (clamp tiny negatives like the reference's maximum(d2,0)).

Exactness: all comparisons use the exact f32 -d2 values; ties resolve to the
lowest index first (hardware max_index returns first occurrences in order),
matching the reference's top_k tie-breaking.
"""

import numpy as np

S = 4096          # points per segment
D = 4             # coordinate dims
B = 8             # segments / cores
K1 = 65           # neighbours incl. self
P = 128           # partitions
NT = S // P       # 32 row tiles
GW = 64           # group width (columns per group)
NG = S // GW      # 64 groups
M_PER_G = 8       # survivors kept per group (one max8, no match_replace round)
CW = NG * M_PER_G # candidate array width (512)
NR = 9            # extraction rounds (9*8 = 72 >= 65)
RW = NR * 8       # 72
NEG_BIG = -3.0e38 # "minus infinity" replacement value (finite for sim checks)

_NC_CACHE = {}


def _build_nc():
    import concourse.bacc as bacc
    import concourse.mybir as mybir
    from concourse import bass
    from concourse.tile import TileContext

    fp32 = mybir.dt.float32
    i16 = mybir.dt.int16
    u16 = mybir.dt.uint16
    i32 = mybir.dt.int32
    Alu = mybir.AluOpType
    Act = mybir.ActivationFunctionType

    nc = bacc.Bacc(None, target_bir_lowering=False, debug=False)

    coords = nc.dram_tensor("coords", [S, D], fp32, kind="ExternalInput")
    out_dist = nc.dram_tensor("out_dist", [S, K1], fp32, kind="ExternalOutput")
    out_idx = nc.dram_tensor("out_idx", [S, K1], i32, kind="ExternalOutput")

    with TileContext(nc) as tc:
        with (
            tc.tile_pool(name="const", bufs=1) as cpool,
            tc.tile_pool(name="nk", bufs=2) as nkpool,
            tc.tile_pool(name="cand", bufs=2) as candpool,
            tc.tile_pool(name="small", bufs=3) as spool,
            tc.tile_pool(name="outs", bufs=3) as opool,
            tc.tile_pool(name="psum", bufs=2, space="PSUM") as ppool,
            tc.tile_pool(name="psumT", bufs=3, space="PSUM") as ptpool,
        ):
            # ---------------- persistent tensors ----------------
            rhs8 = cpool.tile([8, S], fp32)     # rows 0-3: c^T, rows 4-7: (c^T)^2
            lhsT8 = cpool.tile([8, S], fp32)    # rows 0-3: 2*c^T, rows 4-7: -1
            ident = cpool.tile([P, P], fp32)    # identity for PE transpose
            goff = cpool.tile([P, CW], i16)     # 128*(q//16) per C slot
            kio1 = cpool.tile([P, RW], i16)     # k+1
            sqr_all = cpool.tile([P, NT], fp32) # -|c_r|^2 per row, per tile col

            # identity matrix: ones masked to the diagonal
            nc.vector.memset(ident, 1.0)
            nc.gpsimd.affine_select(
                ident, ident, [[1, P]], Alu.is_equal, 0.0,
                base=0, channel_multiplier=-1,
            )
            nc.gpsimd.iota(goff, [[GW, NG], [0, M_PER_G]], base=0,
                           channel_multiplier=0)
            nc.gpsimd.iota(kio1, [[1, RW]], base=1, channel_multiplier=0)
            # rows 4-7 must stay -1; rows 0-3 are overwritten per tile below
            # (engine APs must start at partition 0 mod 32, so fill everything)
            nc.vector.memset(lhsT8, -1.0)

            # ---------------- prologue: build c^T layout ----------------
            for t in range(NT):
                ct8 = spool.tile([P, 2 * D], fp32, tag="ct8")
                # cols 0-3 <- coords rows, cols 4-7 <- squares
                nc.sync.dma_start(ct8[:, 0:D], coords[t * P:(t + 1) * P, :])
                nc.scalar.activation(ct8[:, D:2 * D], ct8[:, 0:D], Act.Square)
                # -|c_r|^2 for this tile's 128 rows
                nc.vector.tensor_reduce(
                    sqr_all[:, t:t + 1], ct8[:, D:2 * D],
                    axis=mybir.AxisListType.X, op=Alu.add, negate=True,
                )
                # transpose [128, 8] -> [8, 128]
                pT = ptpool.tile([2 * D, P], fp32, tag="pT")
                nc.tensor.transpose(pT, ct8, ident)
                cs = slice(t * P, (t + 1) * P)
                nc.scalar.activation(rhs8[:, cs], pT, Act.Copy)
                nc.scalar.activation(lhsT8[0:D, cs], pT[0:D, :], Act.Copy,
                                     scale=2.0)

            # local_scatter lives in gpsimd ucode library 7; load it once
            # (prologue iota/affine_select run in the default library).
            from concourse import library_config
            nc.gpsimd.load_library(library_config.local_scatter)

            # ---------------- main loop over row tiles ----------------
            HB = 1024               # psum half-block columns
            for t in range(NT):
                cs = slice(t * P, (t + 1) * P)
                negkey = nkpool.tile([P, S], fp32, tag="negkey")
                for h in range(S // HB):
                    pshalf = ppool.tile([P, HB], fp32, tag="pshalf")
                    for m in range(HB // 512):
                        col0 = h * HB + m * 512
                        nc.tensor.matmul(
                            pshalf[:, m * 512:(m + 1) * 512],
                            lhsT8[:, cs],
                            rhs8[:, col0:col0 + 512],
                            start=True, stop=True,
                        )
                    # negkey = psum - |c_r|^2   (Identity supports AP bias)
                    nc.scalar.activation(
                        negkey[:, h * HB:(h + 1) * HB], pshalf,
                        Act.Identity, bias=sqr_all[:, t:t + 1],
                    )

                # ---- group phase: top-8 of each 64-wide group ----
                # P(a 64-cell holds >8 of the row's true top-65) ~ 1.3e-6,
                # i.e. ~3 expected rows per full 32768-row run — far below the
                # fp32 rounding noise floor vs the reference (~400 rows).
                Cv = candpool.tile([P, CW], fp32, tag="Cv")
                Cl = candpool.tile([P, CW], u16, tag="Cl")
                for g in range(NG):
                    gs = negkey[:, g * GW:(g + 1) * GW]
                    c0 = g * M_PER_G
                    nc.vector.max(Cv[:, c0:c0 + 8], gs)
                    nc.vector.max_index(Cl[:, c0:c0 + 8], Cv[:, c0:c0 + 8], gs)

                # ---- C phase: global top-72 of the 512 candidates ----
                V = spool.tile([P, RW], fp32, tag="V")
                posC = spool.tile([P, RW], u16, tag="posC")
                for r in range(NR):
                    v8 = V[:, r * 8:(r + 1) * 8]
                    nc.vector.max(v8, Cv)
                    nc.vector.max_index(posC[:, r * 8:(r + 1) * 8], v8, Cv)
                    if r + 1 < NR:
                        nc.vector.match_replace(Cv, v8, Cv, NEG_BIG)

                # ---- index unscramble (GpSimd per-partition scatters) ----
                Cjg = spool.tile([P, CW], i16, tag="Cjg")
                nc.vector.tensor_tensor(
                    out=Cjg, in0=Cl.bitcast(i16), in1=goff, op=Alu.add,
                )
                W2 = spool.tile([P, CW], i16, tag="W2")
                nc.gpsimd.local_scatter(
                    W2, kio1, posC.bitcast(i16),
                    channels=P, num_elems=CW, num_idxs=RW,
                )
                W2m = spool.tile([P, CW], i16, tag="W2m")
                nc.vector.tensor_scalar_add(W2m, W2, -1)
                Y = spool.tile([P, 80], i16, tag="Y")
                nc.gpsimd.local_scatter(
                    Y, Cjg, W2m,
                    channels=P, num_elems=80, num_idxs=CW,
                )

                # ---- outputs ----
                dist65 = opool.tile([P, K1], fp32, tag="dist65")
                idx65 = opool.tile([P, K1], i32, tag="idx65")
                nc.scalar.activation(dist65, V[:, :K1], Act.Relu, scale=-1.0)
                nc.vector.tensor_copy(idx65, Y[:, :K1])
                nc.sync.dma_start(out_dist[cs, :], dist65)
                nc.sync.dma_start(out_idx[cs, :], idx65)

    nc.finalize()
    return nc


def _get_nc():
    if "nc" not in _NC_CACHE:
        _NC_CACHE["nc"] = _build_nc()
    return _NC_CACHE["nc"]


def _numpy_fallback(coordinates, row_splits):
    """Pure-numpy replica of the reference (used only on unexpected shapes)."""
    nB = int(row_splits.shape[0] - 1)
    N, nD = coordinates.shape
    nS = N // nB
    c = coordinates.reshape(nB, nS, nD).astype(np.float32)
    sq = np.sum(c * c, axis=-1)
    d2 = sq[:, :, None] + sq[:, None, :] - 2.0 * np.einsum(
        "bsd,btd->bst", c, c)
    d2 = np.maximum(d2, 0.0).astype(np.float32)
    k1 = min(K1, nS)
    idx = np.argsort(d2, axis=-1, kind="stable")[:, :, :k1]
    dist = np.take_along_axis(d2, idx, axis=-1)
    idx = idx + (np.arange(nB, dtype=np.int32) * nS)[:, None, None]
    return (idx.reshape(N, k1).astype(np.int32),
            dist.reshape(N, k1).astype(np.float32))


def kernel(coordinates, row_splits):
    coordinates = np.ascontiguousarray(coordinates, dtype=np.float32)
    rs = np.asarray(row_splits)
    expected_rs = np.arange(B + 1, dtype=np.int64) * S
    if coordinates.shape != (B * S, D) or rs.shape != (B + 1,) or \
            not np.array_equal(rs.astype(np.int64), expected_rs):
        return _numpy_fallback(coordinates, rs)

    from concourse import bass_utils

    nc = _get_nc()
    in_maps = [
        {"coords": coordinates[b * S:(b + 1) * S]} for b in range(B)
    ]
    res = bass_utils.run_bass_kernel_spmd(nc, in_maps, core_ids=list(range(B)))
    idx = np.concatenate(
        [res.results[b]["out_idx"] + np.int32(b * S) for b in range(B)], axis=0
    ).astype(np.int32)
    dist = np.concatenate(
        [res.results[b]["out_dist"] for b in range(B)], axis=0
    ).astype(np.float32)
    return idx, dist



# revision 6
# speedup vs baseline: 1.6869x; 1.6869x over previous
"""Trainium2 Bass kernel v2 for intra-segment KNN (K=64 neighbours + self).

Problem: coordinates [32768, 4] f32 split into 8 equal segments (events) of
4096 points; per point, find the 65 nearest points (incl. self) within its
segment, returning (idx int32 [32768,65], dist f32 [32768,65]) sorted by
ascending squared distance.

Sharding: one event per NeuronCore (8 cores), pure data parallel.

v2 redesign vs baseline (1.09 ms): the baseline was 100% VectorE-bound
(Max 400us + MaxIndex 400us + MatchReplace 100us).  v2 removes the
per-group MaxIndex pass entirely by embedding the 7-bit within-group
column offset into the low mantissa bits of the fp32 key:

  ekey[r,j] = (negkey[r,j] & 0xFFFFFF80) | (j % 128)      (bitwise, on GpSimd)

where negkey[r,j] = 2*c_r.c_j - |c_j|^2 (TensorE, contraction dim 8; the
per-row -|c_r|^2 term is constant per row and dropped from the selection
key, applied only to the 65 output distances).  Clearing 7 mantissa bits
perturbs each key by <= 2^-17 relative -- far below the correctness gate.
For negative fp32, OR-ing offset bits makes the value slightly more
negative, so equal-bin ties resolve to the lower offset first, matching
the reference's lower-index-first tie-break within a group.

Per-core pipeline (S=4096 points, 32 row tiles of 128):
  PE     : 8 matmuls -> PSUM [128,1024] halves          (~1.7 us/tile)
  ScalarE: copy PSUM -> SBUF fp32                        (~4 us/tile)
  GpSimd : ekey = (nk & ~0x7F) | offpat  (one STT op)    (~6 us/tile)
  VectorE: 32x Max8 over 128-wide groups -> C [128,256]  (~6 us/tile)
           9x (Max8 + MaxIndex) + 8x MatchReplace on C   (~8.5 us/tile)
  VectorE: idx = ((posC>>3)<<7) | (V & 0x7F)             (3 small ops)
  ScalarE: dist = Relu(-V + |c_r|^2)
"""

import numpy as np

S = 4096          # points per segment
D = 4             # coordinate dims
B = 8             # segments / cores
K1 = 65           # neighbours incl. self
P = 128           # partitions
NT = S // P       # 32 row tiles
GW = 128          # group width (columns per group)
NG = S // GW      # 32 groups
M_PER_G = 8       # survivors kept per group
CW = NG * M_PER_G # candidate array width (256)
NR = 9            # extraction rounds (9*8 = 72 >= 65)
RW = NR * 8       # 72
NEG_BIG = -3.0e38 # "minus infinity" replacement value

_NC_CACHE = {}


def _build_nc(nt=NT):
    import concourse.bacc as bacc
    import concourse.mybir as mybir
    from concourse import bass
    from concourse.tile import TileContext

    fp32 = mybir.dt.float32
    fp32r = mybir.dt.float32r
    i32 = mybir.dt.int32
    u32 = mybir.dt.uint32
    Alu = mybir.AluOpType
    Act = mybir.ActivationFunctionType

    nc = bacc.Bacc(None, target_bir_lowering=False, debug=False)

    coords = nc.dram_tensor("coords", [S, D], fp32, kind="ExternalInput")
    out_dist = nc.dram_tensor("out_dist", [nt * P, K1], fp32,
                              kind="ExternalOutput")
    out_idx = nc.dram_tensor("out_idx", [nt * P, K1], i32,
                             kind="ExternalOutput")

    with TileContext(nc) as tc:
        with (
            tc.tile_pool(name="const", bufs=1) as cpool,
            tc.tile_pool(name="nk", bufs=2) as nkpool,
            tc.tile_pool(name="cand", bufs=2) as candpool,
            tc.tile_pool(name="small", bufs=3) as spool,
            tc.tile_pool(name="outs", bufs=3) as opool,
            tc.tile_pool(name="psum", bufs=2, space="PSUM") as ppool,
            tc.tile_pool(name="psumT", bufs=3, space="PSUM") as ptpool,
        ):
            # ---------------- persistent tensors ----------------
            rhs8 = cpool.tile([8, S], fp32)     # rows 0-3: c^T, rows 4-7: (c^T)^2
            lhsT8 = cpool.tile([8, S], fp32)    # rows 0-3: 2*c^T, rows 4-7: -1
            offpat = cpool.tile([P, GW], i32)   # 0..GW-1 (broadcast over groups)
            ct_all = cpool.tile([P, NT * D], fp32)  # coords tile-major
            sq_all = cpool.tile([P, NT * D], fp32)  # squares tile-major
            sqr_pos = cpool.tile([P, NT], fp32) # +|c_r|^2 per row, per tile col
            sqr_neg = cpool.tile([P, NT], fp32) # -|c_r|^2 per row, per tile col

            nc.vector.memset(lhsT8, -1.0)   # DVE is idle during the prologue
            nc.gpsimd.iota(offpat, [[1, GW]], base=0, channel_multiplier=0)
            # int32 scalar constants (bitvec ops need int-typed operands)
            cm128 = cpool.tile([P, 1], i32)   # 0xFFFFFF80
            c127 = cpool.tile([P, 1], i32)    # 0x0000007F
            nc.gpsimd.memset(cm128, -128)
            nc.gpsimd.memset(c127, 127)

            # ---------------- prologue (bulk, no PE transposes) ----------
            # rhs8 rows 0-3 <- coords^T via transpose DMA (AP swap), chunked
            # so the first matmuls can start early
            sq4 = cpool.tile([D, S], fp32)
            PCH = 1024
            dma_queues = [nc.sync, nc.scalar]
            for ch in range(S // PCH):
                cc = slice(ch * PCH, (ch + 1) * PCH)
                q = dma_queues[ch % len(dma_queues)]
                q.dma_start(rhs8[0:D, cc],
                            coords[cc, :].rearrange("a b -> b a"))
                nc.scalar.activation(sq4[:, cc], rhs8[0:D, cc], Act.Square)
                q.dma_start(rhs8[D:2 * D, cc], sq4[:, cc])
                # 2*c^T on DVE (idle during prologue; keeps ScalarE short)
                nc.vector.tensor_scalar_mul(lhsT8[0:D, cc], rhs8[0:D, cc], 2.0)
            # per-row |c_r|^2 in [128, NT] layout via tile-major copy
            nc.scalar.dma_start(
                ct_all.rearrange("p (t c) -> p t c", c=D),
                coords.rearrange("(t p) c -> p t c", p=P))
            nc.scalar.activation(sq_all, ct_all, Act.Square)
            nc.vector.tensor_reduce(
                sqr_pos.unsqueeze(-1),
                sq_all.rearrange("p (t c) -> p t c", c=D),
                axis=mybir.AxisListType.X, op=Alu.add,
            )
            nc.vector.tensor_scalar_mul(sqr_neg, sqr_pos, -1.0)

            # ---------------- main loop over row tiles ----------------
            HB = 1024               # psum half-block columns
            for t in range(nt):
                cs = slice(t * P, (t + 1) * P)
                nk = nkpool.tile([P, S], fp32, tag="nk")
                for h in range(S // HB):
                    pshalf = ppool.tile([P, HB], fp32, tag="pshalf")
                    for m in range(HB // 512):
                        col0 = h * HB + m * 512
                        nc.tensor.matmul(
                            pshalf[:, m * 512:(m + 1) * 512],
                            lhsT8[:, cs],
                            rhs8[:, col0:col0 + 512],
                            start=True, stop=True,
                        )
                    # nk = psum - |c_r|^2 = -d2 (key quantum tracks d2)
                    nc.scalar.activation(
                        nk[:, h * HB:(h + 1) * HB], pshalf,
                        Act.Identity, bias=sqr_neg[:, t:t + 1],
                    )

                # ---- embed 7-bit column offset into low mantissa bits ----
                # ekey = (nk & 0xFFFFFF80) | (j % GW)   (in-place, int32 view)
                # (bitwise ops only exist on DVE; one fused STT pass)
                nki = nk.bitcast(i32)
                offb = offpat.unsqueeze(1).broadcast_to((P, NG, GW))
                nc.vector.scalar_tensor_tensor(
                    nki.rearrange("p (g w) -> p g w", w=GW),
                    nki.rearrange("p (g w) -> p g w", w=GW),
                    cm128, offb,
                    op0=Alu.bitwise_and, op1=Alu.bitwise_or,
                )

                # ---- group phase: top-8 of each 128-wide group ----
                Cv = candpool.tile([P, CW], fp32, tag="Cv")
                for g in range(NG):
                    nc.vector.max(Cv[:, g * M_PER_G:g * M_PER_G + 8],
                                  nk[:, g * GW:(g + 1) * GW])

                # ---- C phase: global top-72 of the 256 candidates ----
                V = spool.tile([P, RW], fp32, tag="V")
                posC = spool.tile([P, RW], u32, tag="posC")
                for r in range(NR):
                    v8 = V[:, r * 8:(r + 1) * 8]
                    nc.vector.max(v8, Cv)
                    nc.vector.max_index(posC[:, r * 8:(r + 1) * 8], v8, Cv)
                    if r + 1 < NR:
                        nc.vector.match_replace(Cv, v8, Cv, NEG_BIG)

                # ---- decode indices: j = ((posC>>3)<<7) | (V & 0x7F) ----
                g128 = spool.tile([P, RW], i32, tag="g128")
                nc.vector.tensor_scalar(
                    g128, posC.bitcast(i32), 3, 7,
                    op0=Alu.logical_shift_right, op1=Alu.logical_shift_left,
                )
                idx72 = opool.tile([P, RW], i32, tag="idx72")
                nc.vector.scalar_tensor_tensor(
                    idx72, V.bitcast(i32), c127, g128,
                    op0=Alu.bitwise_and, op1=Alu.bitwise_or,
                )

                # ---- outputs ----
                dist65 = opool.tile([P, K1], fp32, tag="dist65")
                nc.scalar.activation(dist65, V[:, :K1], Act.Relu, scale=-1.0)
                nc.sync.dma_start(out_dist[cs, :], dist65)
                nc.sync.dma_start(out_idx[cs, :], idx72[:, :K1])

    nc.finalize()
    return nc


def _get_nc():
    if "nc" not in _NC_CACHE:
        _NC_CACHE["nc"] = _build_nc()
    return _NC_CACHE["nc"]


def _numpy_fallback(coordinates, row_splits):
    """Pure-numpy replica of the reference (used only on unexpected shapes)."""
    nB = int(row_splits.shape[0] - 1)
    N, nD = coordinates.shape
    nS = N // nB
    c = coordinates.reshape(nB, nS, nD).astype(np.float32)
    sq = np.sum(c * c, axis=-1)
    d2 = sq[:, :, None] + sq[:, None, :] - 2.0 * np.einsum(
        "bsd,btd->bst", c, c)
    d2 = np.maximum(d2, 0.0).astype(np.float32)
    k1 = min(K1, nS)
    idx = np.argsort(d2, axis=-1, kind="stable")[:, :, :k1]
    dist = np.take_along_axis(d2, idx, axis=-1)
    idx = idx + (np.arange(nB, dtype=np.int32) * nS)[:, None, None]
    return (idx.reshape(N, k1).astype(np.int32),
            dist.reshape(N, k1).astype(np.float32))


def kernel(coordinates, row_splits):
    coordinates = np.ascontiguousarray(coordinates, dtype=np.float32)
    rs = np.asarray(row_splits)
    expected_rs = np.arange(B + 1, dtype=np.int64) * S
    if coordinates.shape != (B * S, D) or rs.shape != (B + 1,) or \
            not np.array_equal(rs.astype(np.int64), expected_rs):
        return _numpy_fallback(coordinates, rs)

    from concourse import bass_utils

    nc = _get_nc()
    in_maps = [
        {"coords": coordinates[b * S:(b + 1) * S]} for b in range(B)
    ]
    res = bass_utils.run_bass_kernel_spmd(nc, in_maps, core_ids=list(range(B)))
    idx = np.concatenate(
        [res.results[b]["out_idx"] + np.int32(b * S) for b in range(B)], axis=0
    ).astype(np.int32)
    dist = np.concatenate(
        [res.results[b]["out_dist"] for b in range(B)], axis=0
    ).astype(np.float32)
    return idx, dist


# revision 7
# speedup vs baseline: 1.7052x; 1.0109x over previous
"""Trainium2 Bass kernel v2 for intra-segment KNN (K=64 neighbours + self).

Problem: coordinates [32768, 4] f32 split into 8 equal segments (events) of
4096 points; per point, find the 65 nearest points (incl. self) within its
segment, returning (idx int32 [32768,65], dist f32 [32768,65]) sorted by
ascending squared distance.

Sharding: one event per NeuronCore (8 cores), pure data parallel.

v2 redesign vs baseline (1.09 ms): the baseline was 100% VectorE-bound
(Max 400us + MaxIndex 400us + MatchReplace 100us).  v2 removes the
per-group MaxIndex pass entirely by embedding the 7-bit within-group
column offset into the low mantissa bits of the fp32 key:

  ekey[r,j] = (negkey[r,j] & 0xFFFFFF80) | (j % 128)      (bitwise, on GpSimd)

where negkey[r,j] = 2*c_r.c_j - |c_j|^2 (TensorE, contraction dim 8; the
per-row -|c_r|^2 term is constant per row and dropped from the selection
key, applied only to the 65 output distances).  Clearing 7 mantissa bits
perturbs each key by <= 2^-17 relative -- far below the correctness gate.
For negative fp32, OR-ing offset bits makes the value slightly more
negative, so equal-bin ties resolve to the lower offset first, matching
the reference's lower-index-first tie-break within a group.

Per-core pipeline (S=4096 points, 32 row tiles of 128):
  PE     : 8 matmuls -> PSUM [128,1024] halves          (~1.7 us/tile)
  ScalarE: copy PSUM -> SBUF fp32                        (~4 us/tile)
  GpSimd : ekey = (nk & ~0x7F) | offpat  (one STT op)    (~6 us/tile)
  VectorE: 32x Max8 over 128-wide groups -> C [128,256]  (~6 us/tile)
           9x (Max8 + MaxIndex) + 8x MatchReplace on C   (~8.5 us/tile)
  VectorE: idx = ((posC>>3)<<7) | (V & 0x7F)             (3 small ops)
  ScalarE: dist = Relu(-V + |c_r|^2)
"""

import numpy as np

S = 4096          # points per segment
D = 4             # coordinate dims
B = 8             # segments / cores
K1 = 65           # neighbours incl. self
P = 128           # partitions
NT = S // P       # 32 row tiles
GW = 128          # group width (columns per group)
NG = S // GW      # 32 groups
M_PER_G = 8       # survivors kept per group
CW = NG * M_PER_G # candidate array width (256)
NR = 9            # extraction rounds (9*8 = 72 >= 65)
RW = NR * 8       # 72
NEG_BIG = -3.0e38 # "minus infinity" replacement value

_NC_CACHE = {}


def _build_nc(nt=NT):
    import concourse.bacc as bacc
    import concourse.mybir as mybir
    from concourse import bass
    from concourse.tile import TileContext

    fp32 = mybir.dt.float32
    fp32r = mybir.dt.float32r
    i32 = mybir.dt.int32
    u32 = mybir.dt.uint32
    Alu = mybir.AluOpType
    Act = mybir.ActivationFunctionType

    nc = bacc.Bacc(None, target_bir_lowering=False, debug=False)

    coords = nc.dram_tensor("coords", [S, D], fp32, kind="ExternalInput")
    out_dist = nc.dram_tensor("out_dist", [nt * P, K1], fp32,
                              kind="ExternalOutput")
    out_idx = nc.dram_tensor("out_idx", [nt * P, K1], i32,
                             kind="ExternalOutput")

    with TileContext(nc) as tc:
        with (
            tc.tile_pool(name="const", bufs=1) as cpool,
            tc.tile_pool(name="nk", bufs=2) as nkpool,
            tc.tile_pool(name="cand", bufs=2) as candpool,
            tc.tile_pool(name="small", bufs=3) as spool,
            tc.tile_pool(name="outs", bufs=3) as opool,
            tc.tile_pool(name="psum", bufs=2, space="PSUM") as ppool,
            tc.tile_pool(name="psumT", bufs=3, space="PSUM") as ptpool,
        ):
            # ---------------- persistent tensors ----------------
            rhs8 = cpool.tile([8, S], fp32)     # rows 0-3: c^T, rows 4-7: (c^T)^2
            lhsT8 = cpool.tile([8, S], fp32)    # rows 0-3: 2*c^T, rows 4-7: -1
            offpat = cpool.tile([P, GW], i32)   # 0..GW-1 (broadcast over groups)
            ct_all = cpool.tile([P, NT * D], fp32)  # coords tile-major
            sq_all = cpool.tile([P, NT * D], fp32)  # squares tile-major
            sqr_pos = cpool.tile([P, NT], fp32) # +|c_r|^2 per row, per tile col
            sqr_neg = cpool.tile([P, NT], fp32) # -|c_r|^2 per row, per tile col

            nc.gpsimd.memset(lhsT8, -1.0)   # Pool is free at t=0
            nc.gpsimd.iota(offpat, [[1, GW]], base=0, channel_multiplier=0)
            # int32 scalar constants (bitvec ops need int-typed operands)
            cm128 = cpool.tile([P, 1], i32)   # 0xFFFFFF80
            c127 = cpool.tile([P, 1], i32)    # 0x0000007F
            nc.gpsimd.memset(cm128, -128)
            nc.gpsimd.memset(c127, 127)

            # ---------------- prologue (bulk, no PE transposes) ----------
            # PE p-state warmup: dummy matmuls keep the tensor engine busy
            # through its 3us clock ramp so tile 0's real matmuls run at
            # full speed
            dum = cpool.tile([8, 512], fp32)
            nc.gpsimd.memset(dum, 1.0)
            for w in range(3):
                pdum = ppool.tile([P, 512], fp32, tag="pdum")
                nc.tensor.matmul(pdum, dum[:, 0:P], dum,
                                 start=True, stop=True)

            # rhs8 rows 0-3 <- coords^T via transpose DMA (AP swap), chunked
            # so the first matmuls can start early
            # per-row |c_r|^2 first: it feeds every tile's ScalarE bias
            nc.scalar.dma_start(
                ct_all.rearrange("p (t c) -> p t c", c=D),
                coords.rearrange("(t p) c -> p t c", p=P))
            nc.scalar.activation(sq_all, ct_all, Act.Square)
            nc.vector.tensor_reduce(
                sqr_pos.unsqueeze(-1),
                sq_all.rearrange("p (t c) -> p t c", c=D),
                axis=mybir.AxisListType.X, op=Alu.add,
            )
            nc.vector.tensor_scalar_mul(sqr_neg, sqr_pos, -1.0)

            sq4 = cpool.tile([D, S], fp32)
            PCH = 1024
            dma_queues = [nc.sync, nc.scalar]
            for ch in range(S // PCH):
                cc = slice(ch * PCH, (ch + 1) * PCH)
                q = dma_queues[ch % len(dma_queues)]
                q.dma_start(rhs8[0:D, cc],
                            coords[cc, :].rearrange("a b -> b a"))
                nc.scalar.activation(sq4[:, cc], rhs8[0:D, cc], Act.Square)
                q.dma_start(rhs8[D:2 * D, cc], sq4[:, cc])
                # 2*c^T on DVE (idle during prologue; keeps ScalarE short)
                nc.vector.tensor_scalar_mul(lhsT8[0:D, cc], rhs8[0:D, cc], 2.0)

            # ---------------- main loop over row tiles ----------------
            HB = 1024               # psum half-block columns
            for t in range(nt):
                cs = slice(t * P, (t + 1) * P)
                nk = nkpool.tile([P, S], fp32, tag="nk")
                for h in range(S // HB):
                    pshalf = ppool.tile([P, HB], fp32, tag="pshalf")
                    for m in range(HB // 512):
                        col0 = h * HB + m * 512
                        nc.tensor.matmul(
                            pshalf[:, m * 512:(m + 1) * 512],
                            lhsT8[:, cs],
                            rhs8[:, col0:col0 + 512],
                            start=True, stop=True,
                        )
                    # nk = psum - |c_r|^2 = -d2 (key quantum tracks d2)
                    nc.scalar.activation(
                        nk[:, h * HB:(h + 1) * HB], pshalf,
                        Act.Identity, bias=sqr_neg[:, t:t + 1],
                    )

                # ---- embed 7-bit column offset into low mantissa bits ----
                # ekey = (nk & 0xFFFFFF80) | (j % GW)   (in-place, int32 view)
                # (bitwise ops only exist on DVE; one fused STT pass)
                nki = nk.bitcast(i32)
                offb = offpat.unsqueeze(1).broadcast_to((P, NG // 2, GW))
                for eh in range(2):
                    es = slice(eh * (S // 2), (eh + 1) * (S // 2))
                    nc.vector.scalar_tensor_tensor(
                        nki[:, es].rearrange("p (g w) -> p g w", w=GW),
                        nki[:, es].rearrange("p (g w) -> p g w", w=GW),
                        cm128, offb,
                        op0=Alu.bitwise_and, op1=Alu.bitwise_or,
                    )

                # ---- group phase: top-8 of each 128-wide group ----
                Cv = candpool.tile([P, CW], fp32, tag="Cv")
                for g in range(NG):
                    nc.vector.max(Cv[:, g * M_PER_G:g * M_PER_G + 8],
                                  nk[:, g * GW:(g + 1) * GW])

                # ---- C phase: global top-72 of the 256 candidates ----
                V = spool.tile([P, RW], fp32, tag="V")
                posC = spool.tile([P, RW], u32, tag="posC")
                for r in range(NR):
                    v8 = V[:, r * 8:(r + 1) * 8]
                    nc.vector.max(v8, Cv)
                    nc.vector.max_index(posC[:, r * 8:(r + 1) * 8], v8, Cv)
                    if r + 1 < NR:
                        nc.vector.match_replace(Cv, v8, Cv, NEG_BIG)

                # ---- decode indices: j = ((posC>>3)<<7) | (V & 0x7F) ----
                g128 = spool.tile([P, RW], i32, tag="g128")
                nc.vector.tensor_scalar(
                    g128, posC.bitcast(i32), 3, 7,
                    op0=Alu.logical_shift_right, op1=Alu.logical_shift_left,
                )
                idx72 = opool.tile([P, RW], i32, tag="idx72")
                nc.vector.scalar_tensor_tensor(
                    idx72, V.bitcast(i32), c127, g128,
                    op0=Alu.bitwise_and, op1=Alu.bitwise_or,
                )

                # ---- outputs ----
                dist65 = opool.tile([P, K1], fp32, tag="dist65")
                nc.scalar.activation(dist65, V[:, :K1], Act.Relu, scale=-1.0)
                nc.sync.dma_start(out_dist[cs, :], dist65)
                nc.sync.dma_start(out_idx[cs, :], idx72[:, :K1])

    nc.finalize()
    return nc


def _get_nc():
    if "nc" not in _NC_CACHE:
        _NC_CACHE["nc"] = _build_nc()
    return _NC_CACHE["nc"]


def _numpy_fallback(coordinates, row_splits):
    """Pure-numpy replica of the reference (used only on unexpected shapes)."""
    nB = int(row_splits.shape[0] - 1)
    N, nD = coordinates.shape
    nS = N // nB
    c = coordinates.reshape(nB, nS, nD).astype(np.float32)
    sq = np.sum(c * c, axis=-1)
    d2 = sq[:, :, None] + sq[:, None, :] - 2.0 * np.einsum(
        "bsd,btd->bst", c, c)
    d2 = np.maximum(d2, 0.0).astype(np.float32)
    k1 = min(K1, nS)
    idx = np.argsort(d2, axis=-1, kind="stable")[:, :, :k1]
    dist = np.take_along_axis(d2, idx, axis=-1)
    idx = idx + (np.arange(nB, dtype=np.int32) * nS)[:, None, None]
    return (idx.reshape(N, k1).astype(np.int32),
            dist.reshape(N, k1).astype(np.float32))


def kernel(coordinates, row_splits):
    coordinates = np.ascontiguousarray(coordinates, dtype=np.float32)
    rs = np.asarray(row_splits)
    expected_rs = np.arange(B + 1, dtype=np.int64) * S
    if coordinates.shape != (B * S, D) or rs.shape != (B + 1,) or \
            not np.array_equal(rs.astype(np.int64), expected_rs):
        return _numpy_fallback(coordinates, rs)

    from concourse import bass_utils

    nc = _get_nc()
    in_maps = [
        {"coords": coordinates[b * S:(b + 1) * S]} for b in range(B)
    ]
    res = bass_utils.run_bass_kernel_spmd(nc, in_maps, core_ids=list(range(B)))
    idx = np.concatenate(
        [res.results[b]["out_idx"] + np.int32(b * S) for b in range(B)], axis=0
    ).astype(np.int32)
    dist = np.concatenate(
        [res.results[b]["out_dist"] for b in range(B)], axis=0
    ).astype(np.float32)
    return idx, dist


# revision 11
# speedup vs baseline: 1.8319x; 1.0743x over previous
"""Trainium2 Bass kernel v2 for intra-segment KNN (K=64 neighbours + self).

Problem: coordinates [32768, 4] f32 split into 8 equal segments (events) of
4096 points; per point, find the 65 nearest points (incl. self) within its
segment, returning (idx int32 [32768,65], dist f32 [32768,65]) sorted by
ascending squared distance.

Sharding: one event per NeuronCore (8 cores), pure data parallel.

v2 redesign vs baseline (1.09 ms): the baseline was 100% VectorE-bound
(Max 400us + MaxIndex 400us + MatchReplace 100us).  v2 removes the
per-group MaxIndex pass entirely by embedding the 7-bit within-group
column offset into the low mantissa bits of the fp32 key:

  ekey[r,j] = (nk[r,j] & 0xFFFFFF80) | (j % 128)

where nk[r,j] = -d2[r,j] = 2*c_r.c_j - |c_j|^2 - |c_r|^2 (TensorE matmul
with contraction dim 8 + per-row ScalarE bias).  Clearing 7 mantissa bits
perturbs each key by <= 2^-17 relative -- far below the correctness gate.
All keys are negative, so OR-ing offset bits makes a value slightly more
negative and equal-bin ties resolve to the lower offset first, matching
the reference's lower-index-first tie-break.  The embed runs as ONE fused
scalar_tensor_tensor (AND+OR) on DVE: bitwise ops exist only on DVE, and
GPSIMD's fp32-internal ALU cannot do exact 31-bit integer arithmetic.

Per-core pipeline (S=4096 points, 32 row tiles of 128):
  PE     : warmup dummies + 8 matmuls -> PSUM halves     (~7 us/tile fp32)
  ScalarE: nk = psum - |c_r|^2 (PSUM -> SBUF, 4 ops)     (~4 us/tile)
  VectorE: embed STT (2 halves)                          (~4.3 us/tile)
           32x Max8 over 128-wide groups -> C [128,256]  (~6.2 us/tile)
           save offsets, re-embed C-slot ids in low 8 bits, then
           9x Max8 + 8x MatchReplace on C (no MaxIndex)  (~6.1 us/tile)
           decode: q = V&0xFF, g = q>>3; per-winner offsets recovered
           via GpSimd double-local_scatter (rank->slot->offset)
  ScalarE: dist = Relu(-V)
VectorE is the bottleneck at ~96% occupancy; the prologue builds
rhs8/lhsT8 via chunked transpose-DMAs (no PE-transpose loop) so the
first tile's matmuls start ~12 us in.
"""

import numpy as np

S = 4096          # points per segment
D = 4             # coordinate dims
B = 8             # segments / cores
K1 = 65           # neighbours incl. self
P = 128           # partitions
NT = S // P       # 32 row tiles
GW = 128          # group width (columns per group)
NG = S // GW      # 32 groups
M_PER_G = 8       # survivors kept per group
CW = NG * M_PER_G # candidate array width (256)
NR = 9            # extraction rounds (9*8 = 72 >= 65)
RW = NR * 8       # 72
NEG_BIG = -3.0e38 # "minus infinity" replacement value

_NC_CACHE = {}


def _build_nc(nt=NT):
    import concourse.bacc as bacc
    import concourse.mybir as mybir
    from concourse import bass
    from concourse.tile import TileContext

    fp32 = mybir.dt.float32
    i16 = mybir.dt.int16
    i32 = mybir.dt.int32
    u32 = mybir.dt.uint32
    Alu = mybir.AluOpType
    Act = mybir.ActivationFunctionType

    nc = bacc.Bacc(None, target_bir_lowering=False, debug=False)

    coords = nc.dram_tensor("coords", [S, D], fp32, kind="ExternalInput")
    out_dist = nc.dram_tensor("out_dist", [nt * P, K1], fp32,
                              kind="ExternalOutput")
    out_idx = nc.dram_tensor("out_idx", [nt * P, K1], i32,
                             kind="ExternalOutput")

    with TileContext(nc) as tc:
        with (
            tc.tile_pool(name="const", bufs=1) as cpool,
            tc.tile_pool(name="nk", bufs=2) as nkpool,
            tc.tile_pool(name="cand", bufs=2) as candpool,
            tc.tile_pool(name="small", bufs=3) as spool,
            tc.tile_pool(name="outs", bufs=3) as opool,
            tc.tile_pool(name="psum", bufs=2, space="PSUM") as ppool,
            tc.tile_pool(name="psumT", bufs=3, space="PSUM") as ptpool,
        ):
            # ---------------- persistent tensors ----------------
            rhs8 = cpool.tile([8, S], fp32)     # rows 0-3: c^T, rows 4-7: (c^T)^2
            lhsT8 = cpool.tile([8, S], fp32)    # rows 0-3: 2*c^T, rows 4-7: -1
            offpat = cpool.tile([P, GW], i32)   # 0..GW-1 (broadcast over groups)
            ct_all = cpool.tile([P, NT * D], fp32)  # coords tile-major
            sq_all = cpool.tile([P, NT * D], fp32)  # squares tile-major
            sqr_pos = cpool.tile([P, NT], fp32) # +|c_r|^2 per row, per tile col
            sqr_neg = cpool.tile([P, NT], fp32) # -|c_r|^2 per row, per tile col

            nc.gpsimd.memset(lhsT8, -1.0)   # Pool is free at t=0
            nc.gpsimd.iota(offpat, [[1, GW]], base=0, channel_multiplier=0)
            # int32 scalar constants (bitvec ops need int-typed operands)
            cm128 = cpool.tile([P, 1], i32)   # 0xFFFFFF80
            cm256 = cpool.tile([P, 1], i32)   # 0xFFFFFF00
            nc.gpsimd.memset(cm128, -128)
            nc.gpsimd.memset(cm256, -256)
            slotpat = cpool.tile([P, CW], i32)  # 0..CW-1
            nc.gpsimd.iota(slotpat, [[1, CW]], base=0, channel_multiplier=0)
            kio1 = cpool.tile([P, RW], i16)     # 1..RW
            nc.gpsimd.iota(kio1, [[1, RW]], base=1, channel_multiplier=0)
            zeros = cpool.tile([P, CW], i32)
            nc.gpsimd.memset(zeros, 0)
            c255 = cpool.tile([P, 1], i32)
            nc.gpsimd.memset(c255, 255)
            c127 = cpool.tile([P, 1], i32)
            nc.gpsimd.memset(c127, 127)
            from concourse import library_config

            # ---------------- prologue (bulk, no PE transposes) ----------
            # PE p-state warmup: dummy matmuls keep the tensor engine busy
            # through its 3us clock ramp so tile 0's real matmuls run at
            # full speed
            dum = cpool.tile([8, 512], fp32)
            nc.gpsimd.memset(dum, 1.0)
            for w in range(3):
                pdum = ppool.tile([P, 512], fp32, tag="pdum")
                nc.tensor.matmul(pdum, dum[:, 0:P], dum,
                                 start=True, stop=True)

            # rhs8 rows 0-3 <- coords^T via transpose DMA (AP swap), chunked
            # so the first matmuls can start early
            # per-row |c_r|^2 first: it feeds every tile's ScalarE bias
            nc.scalar.dma_start(
                ct_all.rearrange("p (t c) -> p t c", c=D),
                coords.rearrange("(t p) c -> p t c", p=P))
            nc.scalar.activation(sq_all, ct_all, Act.Square)
            nc.vector.tensor_reduce(
                sqr_pos.unsqueeze(-1),
                sq_all.rearrange("p (t c) -> p t c", c=D),
                axis=mybir.AxisListType.X, op=Alu.add,
            )
            nc.vector.tensor_scalar_mul(sqr_neg, sqr_pos, -1.0)

            sq4 = cpool.tile([D, S], fp32)
            PCH = 1024
            dma_queues = [nc.sync, nc.scalar]
            for ch in range(S // PCH):
                cc = slice(ch * PCH, (ch + 1) * PCH)
                q = dma_queues[ch % len(dma_queues)]
                q.dma_start(rhs8[0:D, cc],
                            coords[cc, :].rearrange("a b -> b a"))
                nc.scalar.activation(sq4[:, cc], rhs8[0:D, cc], Act.Square)
                q.dma_start(rhs8[D:2 * D, cc], sq4[:, cc])
                # 2*c^T on DVE (idle during prologue; keeps ScalarE short)
                nc.vector.tensor_scalar_mul(lhsT8[0:D, cc], rhs8[0:D, cc], 2.0)

            # local_scatter lives in gpsimd ucode library 7; load it once
            nc.gpsimd.load_library(library_config.local_scatter)

            # ---------------- main loop over row tiles ----------------
            HB = 1024               # psum half-block columns
            for t in range(nt):
                cs = slice(t * P, (t + 1) * P)
                nk = nkpool.tile([P, S], fp32, tag="nk")
                for h in range(S // HB):
                    pshalf = ppool.tile([P, HB], fp32, tag="pshalf")
                    for m in range(HB // 512):
                        col0 = h * HB + m * 512
                        nc.tensor.matmul(
                            pshalf[:, m * 512:(m + 1) * 512],
                            lhsT8[:, cs],
                            rhs8[:, col0:col0 + 512],
                            start=True, stop=True,
                        )
                    # nk = psum - |c_r|^2 = -d2 (key quantum tracks d2)
                    nc.scalar.activation(
                        nk[:, h * HB:(h + 1) * HB], pshalf,
                        Act.Identity, bias=sqr_neg[:, t:t + 1],
                    )

                # ---- embed 7-bit column offset into low mantissa bits ----
                # ekey = (nk & 0xFFFFFF80) | (j % GW)   (in-place, int32 view)
                # (bitwise ops only exist on DVE; one fused STT pass)
                nki = nk.bitcast(i32)
                offb = offpat.unsqueeze(1).broadcast_to((P, NG // 2, GW))
                for eh in range(2):
                    es = slice(eh * (S // 2), (eh + 1) * (S // 2))
                    nc.vector.scalar_tensor_tensor(
                        nki[:, es].rearrange("p (g w) -> p g w", w=GW),
                        nki[:, es].rearrange("p (g w) -> p g w", w=GW),
                        cm128, offb,
                        op0=Alu.bitwise_and, op1=Alu.bitwise_or,
                    )

                # ---- group phase: top-8 of each 128-wide group ----
                Cv = candpool.tile([P, CW], fp32, tag="Cv")
                for g in range(NG):
                    nc.vector.max(Cv[:, g * M_PER_G:g * M_PER_G + 8],
                                  nk[:, g * GW:(g + 1) * GW])

                # ---- save per-slot offsets, re-embed slot ids ----
                # Csave[q] = Cv[q] & 0x7F  (the embedded column offsets)
                Csave32 = spool.tile([P, CW], i32, tag="Csave32")
                nc.vector.scalar_tensor_tensor(
                    Csave32, Cv.bitcast(i32), c127, zeros,
                    op0=Alu.bitwise_and, op1=Alu.bitwise_or,
                )
                Csave = spool.tile([P, CW], i16, tag="Csave")
                nc.vector.tensor_copy(Csave, Csave32)
                # Cv = (Cv & ~0xFF) | q  -- low 8 bits now hold the C slot,
                # so the extraction below needs no MaxIndex at all
                nc.vector.scalar_tensor_tensor(
                    Cv.bitcast(i32), Cv.bitcast(i32), cm256, slotpat,
                    op0=Alu.bitwise_and, op1=Alu.bitwise_or,
                )

                # ---- C phase: top-72 values (slots ride in the low bits) ----
                V = spool.tile([P, RW], fp32, tag="V")
                for r in range(NR):
                    v8 = V[:, r * 8:(r + 1) * 8]
                    nc.vector.max(v8, Cv)
                    if r + 1 < NR:
                        nc.vector.match_replace(Cv, v8, Cv, NEG_BIG)

                # ---- decode: q = V & 0xFF; g128 = (q>>3)<<7;
                #      off = Csave[q] via Pool double-scatter; j = g128+off --
                qwin32 = spool.tile([P, RW], i32, tag="qwin32")
                nc.vector.scalar_tensor_tensor(
                    qwin32, V.bitcast(i32), c255, zeros[:, :RW],
                    op0=Alu.bitwise_and, op1=Alu.bitwise_or,
                )
                qwin = spool.tile([P, RW], i16, tag="qwin")
                nc.vector.tensor_copy(qwin, qwin32)
                g128 = spool.tile([P, RW], i32, tag="g128")
                nc.vector.tensor_scalar(
                    g128, qwin32, 3, 7,
                    op0=Alu.logical_shift_right, op1=Alu.logical_shift_left,
                )
                W2 = spool.tile([P, CW], i16, tag="W2")
                nc.gpsimd.local_scatter(
                    W2, kio1, qwin, channels=P, num_elems=CW, num_idxs=RW)
                W2m = spool.tile([P, CW], i16, tag="W2m")
                nc.vector.tensor_scalar_add(W2m, W2, -1)
                offR = spool.tile([P, RW], i16, tag="offR")
                nc.gpsimd.local_scatter(
                    offR, Csave, W2m, channels=P, num_elems=RW, num_idxs=CW)
                offR32 = spool.tile([P, RW], i32, tag="offR32")
                nc.vector.tensor_copy(offR32, offR)
                idx65 = opool.tile([P, K1], i32, tag="idx65")
                nc.vector.tensor_tensor(out=idx65, in0=g128[:, :K1],
                                        in1=offR32[:, :K1], op=Alu.add)

                # ---- outputs ----
                dist65 = opool.tile([P, K1], fp32, tag="dist65")
                nc.scalar.activation(dist65, V[:, :K1], Act.Relu, scale=-1.0)
                nc.sync.dma_start(out_dist[cs, :], dist65)
                nc.sync.dma_start(out_idx[cs, :], idx65)

    nc.finalize()
    return nc


def _get_nc():
    if "nc" not in _NC_CACHE:
        _NC_CACHE["nc"] = _build_nc()
    return _NC_CACHE["nc"]


def _numpy_fallback(coordinates, row_splits):
    """Pure-numpy replica of the reference (used only on unexpected shapes)."""
    nB = int(row_splits.shape[0] - 1)
    N, nD = coordinates.shape
    nS = N // nB
    c = coordinates.reshape(nB, nS, nD).astype(np.float32)
    sq = np.sum(c * c, axis=-1)
    d2 = sq[:, :, None] + sq[:, None, :] - 2.0 * np.einsum(
        "bsd,btd->bst", c, c)
    d2 = np.maximum(d2, 0.0).astype(np.float32)
    k1 = min(K1, nS)
    idx = np.argsort(d2, axis=-1, kind="stable")[:, :, :k1]
    dist = np.take_along_axis(d2, idx, axis=-1)
    idx = idx + (np.arange(nB, dtype=np.int32) * nS)[:, None, None]
    return (idx.reshape(N, k1).astype(np.int32),
            dist.reshape(N, k1).astype(np.float32))


def kernel(coordinates, row_splits):
    coordinates = np.ascontiguousarray(coordinates, dtype=np.float32)
    rs = np.asarray(row_splits)
    expected_rs = np.arange(B + 1, dtype=np.int64) * S
    if coordinates.shape != (B * S, D) or rs.shape != (B + 1,) or \
            not np.array_equal(rs.astype(np.int64), expected_rs):
        return _numpy_fallback(coordinates, rs)

    from concourse import bass_utils

    nc = _get_nc()
    in_maps = [
        {"coords": coordinates[b * S:(b + 1) * S]} for b in range(B)
    ]
    res = bass_utils.run_bass_kernel_spmd(nc, in_maps, core_ids=list(range(B)))
    idx = np.concatenate(
        [res.results[b]["out_idx"] + np.int32(b * S) for b in range(B)], axis=0
    ).astype(np.int32)
    dist = np.concatenate(
        [res.results[b]["out_dist"] for b in range(B)], axis=0
    ).astype(np.float32)
    return idx, dist
